# revision 26
# baseline (speedup 1.0000x reference)
"""Trainium2 Bass kernel: ConditionedLatentSDETransition, 8-core data-parallel SPMD.

Strategy
--------
Data-parallel over the batch axis (4096 rows -> 512 rows/core). All weights
replicated. Activations are kept TRANSPOSED on-chip ([feature, batch]) so that

  * each MLP layer is a plain PE matmul accumulation over 128-row K chunks,
  * BatchNorm statistics are free-axis reductions (per-partition = per-feature),
  * BN batch statistics are made exact by an 8-core AllReduce of the per-core
    [sum, sumsq] vectors (2 collectives per Euler step, 8KB each).

Host-side prep: all inputs are pre-transposed / pre-scaled with numpy:
  zT = z.T, epsT = (eps*sqrt_h).transpose(0,2,1), utdT = (ut*dt).T, W*T = W.T,
  per-partition bias/gain vectors reshaped to [128, n_tiles].

Matmuls run as float32r (full-rate fp32 path on the PE; plain fp32 runs 4x
slower).  Everything feeding a matmul is declared float32r end-to-end (the BIR
verifier requires producers of fp32r-consumed tensors to round to fp32r).
Set BASSK_MM_DTYPE=fp32 to fall back to exact-fp32 matmuls.

Engine balance: PE is the roofline (~160 MMs/step). PSUM evictions + BN sumsq
run on DVE (tensor_tensor_reduce with free running-sum accumulators); BN apply
/ tanh / exp / ln run on ACT; the Euler update runs on DVE reading f straight
from PSUM.
"""

import os
import numpy as np
from contextlib import ExitStack

import concourse.bass as bass
import concourse.bacc as bacc
import concourse.tile as tile
import concourse.mybir as mybir
from concourse import bass_utils

FP = mybir.dt.float32
AF = mybir.ActivationFunctionType
OP = mybir.AluOpType

N_CORES = 8
B, D, U, NOBS = 4096, 512, 64, 25
H1 = H2 = 1024
HD = 512
NE = 8
R = B // N_CORES          # 512 rows per core
P = 128
BN_EPS = 1e-5

nD, nH1, nH2, nHD = D // P, H1 // P, H2 // P, HD // P  # 4, 8, 8, 4

_MM_DT = {
    "fp32r": mybir.dt.float32r,
    "fp32": mybir.dt.float32,
}[os.environ.get("BASSK_MM_DTYPE", "fp32r")]


def build(h_val, repeats=1, collectives=True, n_cores=N_CORES,
          diffusion_first=False, use_ag=False, n_steps=NE):
    """Build + compile the bass program.  `repeats` re-runs the whole
    computation (timing amplification only; results valid for repeats=1).
    diffusion_first: emit the diffusion block right after the L1 stats send
    (higher priority) instead of after the BN2 applies."""
    MF = _MM_DT
    nc = bacc.Bacc("TRN2", target_bir_lowering=False, debug=False,
                   num_devices=n_cores)

    def din(name, shape, dt=FP):
        return nc.dram_tensor(name, list(shape), dt, kind="ExternalInput").ap()

    # fp32r-typed inputs (feed matmuls); numpy repr of float32r is float32
    zT_d = din("zT", (D, R), MF)
    utdT_d = din("utdT", (U, R), MF)
    w1t_d = din("w1t", (D, H1), MF)
    w2t_d = din("w2t", (H1, H2), MF)
    w3t_d = din("w3t", (H2, D), MF)
    wd1t_d = din("wd1t", (D, HD), MF)
    wd2t_d = din("wd2t", (HD, D), MF)
    bsdet_d = din("bsdet", (U, D), MF)
    ct_d = din("ct", (D, NOBS), MF)
    dtt_d = din("dtt", (U, NOBS), MF)
    # fp32 inputs (never consumed by a matmul)
    epsT_d = din("epsT", (NE, D, R))
    g1_d = din("g1m", (P, nH1))
    be1_d = din("be1m", (P, nH1))
    g2_d = din("g2m", (P, nH2))
    be2_d = din("be2m", (P, nH2))
    bd1_d = din("bd1m", (P, nHD))
    bd2_d = din("bd2m", (P, nHD))
    hb3_d = din("hb3m", (P, nD))

    znext_d = nc.dram_tensor("znextT", [D, R], FP, kind="ExternalOutput").ap()
    yt_d = nc.dram_tensor("ytT", [NOBS, R], FP, kind="ExternalOutput").ap()

    with tile.TileContext(nc) as tc, ExitStack() as ctx:
        wp = ctx.enter_context(tc.tile_pool(name="w", bufs=1))
        sp = ctx.enter_context(tc.tile_pool(name="state", bufs=1))
        wk = ctx.enter_context(tc.tile_pool(name="work", bufs=2))
        ep_p = ctx.enter_context(tc.tile_pool(name="epsp", bufs=2))
        ps_p = ctx.enter_context(tc.tile_pool(name="psum", bufs=6, space="PSUM"))
        psy_p = ctx.enter_context(tc.tile_pool(name="psumy", bufs=1, space="PSUM"))
        dr_p = ctx.enter_context(tc.tile_pool(name="drp", bufs=2, space="DRAM"))

        def load_rows(dap, n_tiles, width, tagp, dt=MF):
            ts = []
            for kk in range(n_tiles):
                t = wp.tile([P, width], dt, tag=f"{tagp}{kk}", name=f"{tagp}{kk}")
                nc.sync.dma_start(t[:], dap[kk * P:(kk + 1) * P, :])
                ts.append(t)
            return ts

        def load_one(dap, shape, tag, dt=MF):
            t = wp.tile(list(shape), dt, tag=tag, name=tag)
            nc.sync.dma_start(t[:], dap[:])
            return t

        # weights, in first-use order (DMA priority follows emission order)
        w1 = load_rows(w1t_d, nD, H1, "w1")
        wd1 = load_rows(wd1t_d, nD, HD, "wd1")
        wd2 = load_rows(wd2t_d, nHD, D, "wd2")
        w2 = load_rows(w2t_d, nH1, H2, "w2")
        w3 = load_rows(w3t_d, nH2, D, "w3")
        bsdet = load_one(bsdet_d, (U, D), "bsdet")
        ct = load_rows(ct_d, nD, NOBS, "ct")
        dtt = load_one(dtt_d, (U, NOBS), "dtt")
        g1m = load_one(g1_d, (P, nH1), "g1m", FP)
        be1m = load_one(be1_d, (P, nH1), "be1m", FP)
        g2m = load_one(g2_d, (P, nH2), "g2m", FP)
        be2m = load_one(be2_d, (P, nH2), "be2m", FP)
        bd1m = load_one(bd1_d, (P, nHD), "bd1m", FP)
        bd2m = load_one(bd2_d, (P, nHD), "bd2m", FP)
        hb3m = load_one(hb3_d, (P, nD), "hb3m", FP)
        utd = load_one(utdT_d, (U, R), "utd")
        epsc = wp.tile([P, 1], FP, tag="epsc", name="epsc")
        nc.vector.memset(epsc[:], BN_EPS)

        # persistent state ([feature, local-batch]); matmul inputs are fp32r
        z = [sp.tile([P, R], MF, tag=f"z{j}", name=f"z{j}") for j in range(nD)]
        a1 = [sp.tile([P, R], MF, tag=f"a1_{j}", name=f"a1_{j}") for j in range(nH1)]
        a2 = [sp.tile([P, R], MF, tag=f"a2_{j}", name=f"a2_{j}") for j in range(nH2)]
        tt = [sp.tile([P, R], MF, tag=f"t{j}", name=f"t{j}") for j in range(nHD)]
        gg = [sp.tile([P, R], FP, tag=f"g{j}", name=f"g{j}") for j in range(nD)]
        for j in range(nD):
            nc.sync.dma_start(z[j][:], zT_d[j * P:(j + 1) * P, :])

        def layer_mm_stats(w_tiles, in_tiles, out_tiles, st, stag):
            """out_tiles[j] = sum_c w_tiles[c][:, jP:(j+1)P].T @ in_tiles[c].
            Per-tile: bn_stats on the PSUM group (one DVE pass -> [cnt, mean,
            M2] for even/odd elements) + DVE tensor_copy eviction to SBUF.
            Parity stats are then combined into st[:, 0:8] = sum/(R/2) pairs
            (me+mo) and st[:, 8:16] = local sum of x^2, ready for AllReduce."""
            n_out = len(out_tiles)
            n_in = len(in_tiles)
            bst = wk.tile([P, 6 * n_out], FP, tag=f"bst{stag}",
                          name=f"bst{stag}")
            for j in range(n_out):
                ps = ps_p.tile([P, R], FP, tag="ps", name=f"ps{j}")
                for c in range(n_in):
                    nc.tensor.matmul(ps[:], w_tiles[c][:, j * P:(j + 1) * P],
                                     in_tiles[c][:],
                                     start=(c == 0), stop=(c == n_in - 1))
                nc.vector.bn_stats(bst[:, 6 * j:6 * j + 6], ps[:])
                nc.vector.tensor_copy(out_tiles[j][:], ps[:])
            bv = bst[:].rearrange("p (j s) -> p j s", s=6)
            me, m2e = bv[:, :, 1], bv[:, :, 2]
            mo, m2o = bv[:, :, 4], bv[:, :, 5]
            # st[:, 0:8] = me+mo  (global mean = AR-sum * (R/2)/N)
            nc.vector.tensor_add(st[:, 0:n_out], me, mo)
            # st[:, 8:16] = sum(x^2) = M2e+M2o + (R/2)*(me^2+mo^2)
            t1 = wk.tile([P, n_out], FP, tag=f"t1{stag}", name=f"t1{stag}")
            nc.vector.tensor_mul(t1[:], me, me)
            t2 = wk.tile([P, n_out], FP, tag=f"t2{stag}", name=f"t2{stag}")
            nc.vector.tensor_mul(t2[:], mo, mo)
            nc.vector.tensor_add(t1[:], t1[:], t2[:])
            t3 = wk.tile([P, n_out], FP, tag=f"t3{stag}", name=f"t3{stag}")
            nc.vector.tensor_add(t3[:], m2e, m2o)
            nc.vector.scalar_tensor_tensor(st[:, 8:8 + n_out], t1[:],
                                           float(R // 2), t3[:],
                                           OP.mult, OP.add)

        def stats_global(st, tagp):
            """Global [sum, sumsq] stats across cores -> SBUF tile.
            AllReduce, or AllGather + local sum (lower latency floor)."""
            if not collectives:
                return st
            arin = dr_p.tile([P, 16], FP, tag=f"{tagp}i", name=f"{tagp}i")
            nc.sync.dma_start(arin[:], st[:])
            if use_ag:
                ago = dr_p.tile([P * n_cores, 16], FP, tag=f"{tagp}o",
                                name=f"{tagp}o")
                nc.gpsimd.collective_compute(
                    "AllGather", OP.bypass,
                    replica_groups=[list(range(n_cores))],
                    ins=[arin.opt()], outs=[ago.opt()])
                # gather back as [P, 16, n_cores] then reduce the rank axis
                g8 = wk.tile([P, 16 * n_cores], FP, tag=f"{tagp}g8",
                             name=f"{tagp}g8")
                nc.sync.dma_start(
                    g8[:].rearrange("p (s c) -> p s c", c=n_cores),
                    ago[:].rearrange("(c p) s -> p s c", c=n_cores))
                gst = wk.tile([P, 16], FP, tag=f"{tagp}g", name=f"{tagp}g")
                nc.vector.tensor_reduce(
                    gst[:], g8[:].rearrange("p (s c) -> p s c", c=n_cores),
                    mybir.AxisListType.X, OP.add)
                return gst
            arout = dr_p.tile([P, 16], FP, tag=f"{tagp}o", name=f"{tagp}o")
            nc.gpsimd.collective_compute(
                "AllReduce", OP.add,
                replica_groups=[list(range(n_cores))],
                ins=[arin.opt()], outs=[arout.opt()])
            gst = wk.tile([P, 16], FP, tag=f"{tagp}g", name=f"{tagp}g")
            nc.sync.dma_start(gst[:], arout[:])
            return gst

        n_total = B if collectives else R
        inv_n = 1.0 / n_total
        mu_f = float(R // 2) / n_total   # st[:,0:8] holds (me+mo) pairs

        def bn_coeffs(gst, n, gm, bem, tagp):
            """scale/shift [P, n] from global sums: scale = g/sqrt(var+eps),
            shift = be - mu*scale."""
            mu = wk.tile([P, n], FP, tag=f"{tagp}mu", name=f"{tagp}mu")
            nc.vector.tensor_scalar(mu[:], gst[:, 0:n], mu_f, None, OP.mult)
            musq = wk.tile([P, n], FP, tag=f"{tagp}m2", name=f"{tagp}m2")
            nc.vector.tensor_mul(musq[:], mu[:], mu[:])
            var = wk.tile([P, n], FP, tag=f"{tagp}v", name=f"{tagp}v")
            nc.vector.scalar_tensor_tensor(var[:], gst[:, 8:8 + n], inv_n,
                                           musq[:], OP.mult, OP.subtract)
            sd = wk.tile([P, n], FP, tag=f"{tagp}sd", name=f"{tagp}sd")
            nc.scalar.activation(sd[:], var[:], AF.Sqrt, bias=epsc[:, 0:1])
            inv = wk.tile([P, n], FP, tag=f"{tagp}in", name=f"{tagp}in")
            nc.vector.reciprocal(inv[:], sd[:])
            scl = wk.tile([P, n], FP, tag=f"{tagp}sc", name=f"{tagp}sc")
            nc.vector.tensor_mul(scl[:], inv[:], gm[:])
            sft = wk.tile([P, n], FP, tag=f"{tagp}sf", name=f"{tagp}sf")
            nc.vector.scalar_tensor_tensor(sft[:], mu[:], -1.0, scl[:],
                                           OP.mult, OP.mult)
            nc.vector.tensor_add(sft[:], sft[:], bem[:])
            return scl, sft

        for _rep in range(repeats):
            for k in range(n_steps):
                ep = [ep_p.tile([P, R], FP, tag=f"eps{j}", name=f"eps{j}")
                      for j in range(nD)]
                for j in range(nD):
                    nc.sync.dma_start(ep[j][:], epsT_d[k, j * P:(j + 1) * P, :])

                def emit_diff_a():
                    # diffusion layer 1 (tanh) — emitted right after the AR1
                    # send: per-engine instruction streams are STATIC, so this
                    # is what PE/ACT execute during the collective wait.
                    for j in range(nHD):
                        ps = ps_p.tile([P, R], FP, tag="ps", name=f"psd1{j}")
                        for c in range(nD):
                            nc.tensor.matmul(ps[:],
                                             wd1[c][:, j * P:(j + 1) * P],
                                             z[c][:],
                                             start=(c == 0),
                                             stop=(c == nD - 1))
                        nc.scalar.activation(tt[j][:], ps[:], AF.Tanh,
                                             bias=bd1m[:, j:j + 1])

                def emit_diff_b():
                    # diffusion layer 2 (softplus = ln(1+exp), inputs O(1)) —
                    # emitted right after the AR2 send to fill that wait.
                    # All Exp passes then all Ln passes (fewer table loads).
                    for j in range(nD):
                        ps = ps_p.tile([P, R], FP, tag="ps", name=f"psd2{j}")
                        for c in range(nHD):
                            nc.tensor.matmul(ps[:],
                                             wd2[c][:, j * P:(j + 1) * P],
                                             tt[c][:],
                                             start=(c == 0),
                                             stop=(c == nHD - 1))
                        nc.scalar.activation(gg[j][:], ps[:], AF.Exp,
                                             bias=bd2m[:, j:j + 1])
                    for j in range(nD):
                        nc.scalar.activation(gg[j][:], gg[j][:], AF.Ln,
                                             bias=1.0)

                # drift layer 1 + stats
                st1 = wk.tile([P, 16], FP, tag="st1", name="st1")
                layer_mm_stats(w1, z, a1, st1, "1")
                gst1 = stats_global(st1, "ar1")

                emit_diff_a()
                if diffusion_first:
                    emit_diff_b()

                # BN1 apply + drift layer 2
                scl1, sft1 = bn_coeffs(gst1, nH1, g1m, be1m, "bn1")
                for j in range(nH1):
                    nc.scalar.activation(a1[j][:], a1[j][:], AF.Relu,
                                         bias=sft1[:, j:j + 1],
                                         scale=scl1[:, j:j + 1])
                st2 = wk.tile([P, 16], FP, tag="st2", name="st2")
                layer_mm_stats(w2, a1, a2, st2, "2")
                gst2 = stats_global(st2, "ar2")
                if not diffusion_first:
                    emit_diff_b()
                scl2, sft2 = bn_coeffs(gst2, nH2, g2m, be2m, "bn2")
                for j in range(nH2):
                    nc.scalar.activation(a2[j][:], a2[j][:], AF.Relu,
                                         bias=sft2[:, j:j + 1],
                                         scale=scl2[:, j:j + 1])

                # drift layer 3 + Euler-Maruyama update (all on DVE)
                for j in range(nD):
                    ps = ps_p.tile([P, R], FP, tag="ps", name=f"psf{j}")
                    for c in range(nH2):
                        nc.tensor.matmul(ps[:], w3[c][:, j * P:(j + 1) * P],
                                         a2[c][:],
                                         start=(c == 0), stop=(c == nH2 - 1))
                    fs = wk.tile([P, R], FP, tag="fs", name="fs")
                    # fs = h*f_raw + h*b3   (hb3 = h*b3 precomputed on host)
                    nc.vector.tensor_scalar(fs[:], ps[:], float(h_val),
                                            hb3m[:, j:j + 1], OP.mult, OP.add)
                    # g = (softplus + 1e-5) * (sqrt_h * eps)  [eps pre-scaled]
                    nc.vector.scalar_tensor_tensor(gg[j][:], gg[j][:], 1e-5,
                                                   ep[j][:], OP.add, OP.mult)
                    nc.vector.tensor_add(z[j][:], z[j][:], fs[:])
                    nc.vector.tensor_add(z[j][:], z[j][:], gg[j][:])

        # z_next = z + (ut*dt) @ B_sde.T ;  yt = z_next @ C.T + (ut*dt) @ D.T
        for j in range(nD):
            ps = ps_p.tile([P, R], FP, tag="ps", name=f"psb{j}")
            nc.tensor.matmul(ps[:], bsdet[:, j * P:(j + 1) * P], utd[:],
                             start=True, stop=True)
            nc.vector.tensor_add(z[j][:], z[j][:], ps[:])
            nc.sync.dma_start(znext_d[j * P:(j + 1) * P, :],
                              z[j][:].bitcast(FP))
        psy = psy_p.tile([NOBS, R], FP, tag="psy", name="psy")
        for c in range(nD):
            nc.tensor.matmul(psy[:], ct[c][:, :], z[c][:],
                             start=(c == 0), stop=False)
        nc.tensor.matmul(psy[:], dtt[:, :], utd[:],
                         start=False, stop=True)
        yts = wk.tile([NOBS, R], FP, tag="yts", name="yts")
        nc.scalar.activation(yts[:], psy[:], AF.Copy)
        nc.sync.dma_start(yt_d[:], yts[:])

    nc.compile()
    return nc


def prepare_in_maps(inputs):
    """Host-side transforms + per-core sharding.  Returns (in_maps, h)."""
    f32 = lambda k: np.asarray(inputs[k], dtype=np.float32)
    dt_val = np.float32(np.asarray(inputs["dt"]).reshape(-1)[0])
    h = np.float32(dt_val / np.float32(NE))
    sqrt_h = np.float32(np.sqrt(np.abs(h) + np.float32(1e-8)))

    zT = np.ascontiguousarray(f32("z_dyn").T)                       # [D, B]
    epsT = np.ascontiguousarray(
        (f32("eps") * sqrt_h).transpose(0, 2, 1))                   # [NE, D, B]
    utdT = np.ascontiguousarray((f32("ut") * dt_val).T)             # [U, B]

    def colmat(v, n):  # [n*P] -> [P, n], column j = v[j*P:(j+1)*P]
        return np.ascontiguousarray(np.asarray(v, np.float32).reshape(n, P).T)

    shared = {
        "w1t": np.ascontiguousarray(f32("W1").T),
        "w2t": np.ascontiguousarray(f32("W2").T),
        "w3t": np.ascontiguousarray(f32("W3").T),
        "wd1t": np.ascontiguousarray(f32("Wd1").T),
        "wd2t": np.ascontiguousarray(f32("Wd2").T),
        "bsdet": np.ascontiguousarray(f32("B_sde").T),
        "ct": np.ascontiguousarray(f32("C").T),
        "dtt": np.ascontiguousarray(f32("D").T),
        "g1m": colmat(inputs["g1"], nH1),
        "be1m": colmat(inputs["be1"], nH1),
        "g2m": colmat(inputs["g2"], nH2),
        "be2m": colmat(inputs["be2"], nH2),
        "bd1m": colmat(inputs["bd1"], nHD),
        "bd2m": colmat(inputs["bd2"], nHD),
        "hb3m": colmat(f32("b3") * h, nD),
    }
    in_maps = []
    for c in range(N_CORES):
        sl = slice(c * R, (c + 1) * R)
        m = dict(shared)
        m["zT"] = np.ascontiguousarray(zT[:, sl])
        m["epsT"] = np.ascontiguousarray(epsT[:, :, sl])
        m["utdT"] = np.ascontiguousarray(utdT[:, sl])
        in_maps.append(m)
    return in_maps, h


def gather(results):
    z_next = np.concatenate(
        [results[c]["znextT"].T for c in range(N_CORES)], axis=0)
    yt = np.concatenate(
        [results[c]["ytT"].T for c in range(N_CORES)], axis=0)
    return (np.ascontiguousarray(z_next, dtype=np.float32),
            np.ascontiguousarray(yt, dtype=np.float32))


_cache = {}


def _compiled(h, repeats=1):
    key = (float(h), int(repeats), _MM_DT)
    if key not in _cache:
        _cache[key] = build(float(h), repeats=repeats)
    return _cache[key]


def kernel(**inputs):
    in_maps, h = prepare_in_maps(inputs)
    nc = _compiled(h)
    res = bass_utils.run_bass_kernel_spmd(nc, in_maps,
                                          core_ids=list(range(N_CORES)))
    return gather(res.results)
